# revision 1
# baseline (speedup 1.0000x reference)
"""HardNegativeInfoNCELoss on 8 Trainium2 NeuronCores.

Strategy (v2, scan-split redesign):
  * Host: L2-normalize anchor/positive/negative_pool (fp32), scale by 64 and
    quantize to fp8 e4m3 with the K=256 contraction interleaved as 2 k-tiles
    (DoubleRow layout).  Pool columns sharded across 8 cores (M/8 = 32768).
  * Device (SPMD, per core): stream the pool shard chunk-by-chunk (2048 cols);
    for every (128-anchor tile, 1024-col PSUM sub-tile) run 2 fp8 DoubleRow
    matmuls (K=256 in one pass, ~2x bf16 ALU rate).  The B*M/8 sim scan is the
    bottleneck (max8/reduce are always 1 elem/cycle/lane), so it is split:
      - anchor tiles b=0,1: DVE max8 drains PSUM directly (top-8 / 1024 cols)
      - anchor tiles b=2..7: ScalarE copies PSUM -> SBUF fp16 into per-b
        8192-wide concat buffers (phase-staggered across chunks); DVE folds
        them with tensor_max at the 2x 16-bit mode (4096+2048+1024 outputs)
        and a final max8 -> top-8 per 8192-col window.
    This keeps DVE + ScalarE both ~100% busy instead of DVE-only.
  * Host: merge candidates (divide by 64^2), exact top-10 per anchor, append
    the fp32-exact positive logit, evaluate the InfoNCE loss.

  Window/candidate safety: a window of W cols with k=8 kept misses a true
  top-10 member only if >=9 of them land in one window (p ~ 1e-11 for
  W=8192); fold slot-collisions add ~1e-3 lost members per batch, shifting
  the loss by ~2e-6 relative.  fp8 e4m3 sim noise gives ~9e-4 relative loss
  error (measured on host vs fp32 reference), well inside the 2e-2 gate.
"""

import os
import sys

import numpy as np


def _ensure_concourse():
    try:
        import concourse  # noqa: F401
        return
    except ImportError:
        pass
    for p in ("/opt/trn_rl_repo", "/root/.axon_site/_ro/trn_rl_repo"):
        if os.path.isdir(os.path.join(p, "concourse")):
            sys.path.insert(0, p)
            return


_ensure_concourse()

N_CORES = 8
B = 1024
D = 256
M = 262144
M_SHARD = M // N_CORES  # 32768
CHUNK = 2048
N_CHUNKS = M_SHARD // CHUNK  # 16
SUB = 1024  # psum sub-tile columns
NB = B // 128  # 8 anchor tiles
N_A_B = 2  # anchor tiles drained by DVE max8 directly
N_Y_B = NB - N_A_B  # anchor tiles routed through ScalarE+fold lane
PHASE = [0, 1, 2, 3, 0, 2]  # per-Y-b chunk phase for concat slots
GROUPS_PER_B = [5, 5, 6, 6, 5, 6]  # regular + early(c15) + tail groups
Y_CUM = [0, 5, 10, 16, 22, 27]  # cumulative group offsets
A_REGION = N_A_B * N_CHUNKS * 2 * 8  # 512
Y_REGION = sum(GROUPS_PER_B) * 8  # 216
CAND_COLS = A_REGION + Y_REGION  # 728
SCALE = 64.0
TEMPERATURE = 0.07
NUM_HARD_NEGATIVES = 10
EPS = 1e-12

_program = None


def _a_off(bi, c, h):
    return ((bi * N_CHUNKS + c) * 2 + h) * 8


def _y_off(byi, gi):
    return A_REGION + (Y_CUM[byi] + gi) * 8


def _build_program():
    import concourse.bacc as bacc
    import concourse.mybir as mybir
    from concourse.tile import TileContext

    nc = bacc.Bacc(
        "TRN2", target_bir_lowering=False, debug=False, num_devices=N_CORES
    )
    f16 = mybir.dt.float16
    f32 = mybir.dt.float32
    fp8 = mybir.dt.float8e4
    DR = mybir.MatmulPerfMode.DoubleRow
    Copy = mybir.ActivationFunctionType.Copy

    # AT8[p, t*1024 + m] = a8[m, t*128 + p];  PT8[p, c*4096 + t*2048 + j]
    AT8 = nc.dram_tensor("AT8", [128, 2 * B], fp8, kind="ExternalInput")
    PT8 = nc.dram_tensor("PT8", [128, 2 * M_SHARD], fp8, kind="ExternalInput")
    CAND = nc.dram_tensor("CAND", [128, CAND_COLS], f32, kind="ExternalOutput")

    with TileContext(nc) as tc:
        with (
            tc.tile_pool(name="const", bufs=1) as cpool,
            tc.tile_pool(name="stream", bufs=2) as spool,
            tc.tile_pool(name="psum_a", bufs=2, space="PSUM") as apool,
            tc.tile_pool(name="psum_y", bufs=2, space="PSUM") as ypool,
        ):
            at8 = cpool.tile([128, 2 * B], fp8)
            nc.sync.dma_start(out=at8, in_=AT8[:, :])
            a8v = at8[:, :].rearrange("p (t m) -> p t m", t=2)  # [128,2,1024]

            cand = cpool.tile([128, CAND_COLS], f32)
            nc.vector.memset(cand, -1e30)

            concat = [
                cpool.tile([128, 4 * CHUNK], f16, name=f"concat{i}")
                for i in range(N_Y_B)
            ]
            ftmp1 = cpool.tile([128, 4096], f16)
            ftmp2 = cpool.tile([128, 2048], f16)
            ftmp2b = cpool.tile([128, 2048], f16)
            ftmp3 = cpool.tile([128, 1024], f16)
            ftmp4 = cpool.tile([128, 512], f16)

            gi_next = [0] * N_Y_B
            pending = []

            def emit_fold(byi, lo, nslots):
                cc = concat[byi]
                if nslots == 4:
                    nc.vector.tensor_max(ftmp1, cc[:, 0:4096], cc[:, 4096:8192])
                    nc.vector.tensor_max(
                        ftmp2, ftmp1[:, 0:2048], ftmp1[:, 2048:4096]
                    )
                    f2 = ftmp2
                elif nslots == 3:
                    nc.vector.tensor_max(
                        ftmp2, cc[:, lo : lo + 2048], cc[:, lo + 2048 : lo + 4096]
                    )
                    nc.vector.tensor_max(
                        ftmp2b, ftmp2, cc[:, lo + 4096 : lo + 6144]
                    )
                    f2 = ftmp2b
                elif nslots == 2:
                    nc.vector.tensor_max(
                        ftmp2, cc[:, lo : lo + 2048], cc[:, lo + 2048 : lo + 4096]
                    )
                    f2 = ftmp2
                else:  # 1 slot
                    f2 = cc[:, lo : lo + 2048]
                nc.vector.tensor_max(ftmp3, f2[:, 0:1024], f2[:, 1024:2048])
                nc.vector.tensor_max(ftmp4, ftmp3[:, 0:512], ftmp3[:, 512:1024])
                o = _y_off(byi, gi_next[byi])
                gi_next[byi] += 1
                nc.vector.max(out=cand[:, o : o + 8], in_=ftmp4)

            for c in range(N_CHUNKS):
                pt = spool.tile([128, 2 * CHUNK], fp8, tag="pt", name="pt")
                nc.sync.dma_start(
                    out=pt, in_=PT8[:, c * 2 * CHUNK : (c + 1) * 2 * CHUNK]
                )
                ptv = pt[:, :].rearrange("p (t j) -> p t j", t=2)

                # folds whose inputs completed during the previous chunk
                for args in pending:
                    emit_fold(*args)
                pending = []
                if c == N_CHUNKS - 1:
                    # pre-fold each concat's slots filled through chunk 14 so
                    # only a 1-slot fold per b remains after the last copy
                    for byi in range(N_Y_B):
                        s15 = (c + PHASE[byi]) % 4
                        if s15 > 0:
                            emit_fold(byi, 0, s15)

                for b in list(range(N_A_B, NB)) + list(range(N_A_B)):
                    a_sl = a8v[:, :, b * 128 : (b + 1) * 128]
                    for h in range(2):
                        pool = apool if b < N_A_B else ypool
                        ps = pool.tile([128, SUB], f32, tag="ps", name="ps")
                        for n in range(2):
                            col = h * SUB + n * 512
                            nc.tensor.matmul(
                                ps[:, n * 512 : (n + 1) * 512],
                                a_sl,
                                ptv[:, :, col : col + 512],
                                start=True,
                                stop=True,
                                perf_mode=DR,
                            )
                        if b < N_A_B:
                            o = _a_off(b, c, h)
                            nc.vector.max(out=cand[:, o : o + 8], in_=ps)
                        else:
                            byi = b - N_A_B
                            slot = (c + PHASE[byi]) % 4
                            dst = concat[byi][
                                :,
                                slot * CHUNK + h * SUB :
                                slot * CHUNK + h * SUB + SUB,
                            ]
                            nc.scalar.activation(out=dst, in_=ps, func=Copy)
                            if slot == 3 and h == 1 and c < N_CHUNKS - 1:
                                p = PHASE[byi]
                                if gi_next[byi] == 0 and p > 0:
                                    # first group: only (4-p) fresh slots
                                    pending.append((byi, p * CHUNK, 4 - p))
                                else:
                                    pending.append((byi, 0, 4))
            assert not pending
            # tail: fold only the final chunk's slot for every Y-b
            for byi in range(N_Y_B):
                s15 = (N_CHUNKS - 1 + PHASE[byi]) % 4
                emit_fold(byi, s15 * CHUNK, 1)
            nc.sync.dma_start(out=CAND[:, :], in_=cand)
    nc.compile()
    return nc


def _get_program():
    global _program
    if _program is None:
        _program = _build_program()
    return _program


def _normalize_rows(x):
    n = np.sqrt((x.astype(np.float32) ** 2).sum(axis=-1, keepdims=True))
    return x / np.maximum(n, EPS)


def run_device(anchor, negative_pool, trace=False, tmpdir=None):
    """Run the SPMD device program; returns (per-core CAND list, results obj)."""
    from concourse.bass_utils import run_bass_kernel_spmd

    import ml_dtypes

    fp8 = ml_dtypes.float8_e4m3
    a = _normalize_rows(np.asarray(anchor, dtype=np.float32)) * SCALE
    n = _normalize_rows(np.asarray(negative_pool, dtype=np.float32)) * SCALE
    a8 = a.astype(fp8)  # [B, 256]
    n8 = n.astype(fp8)  # [M, 256]

    # AT8[p, t, m] = a8[m, t*128+p]
    at8 = np.ascontiguousarray(
        a8.reshape(B, 2, 128).transpose(2, 1, 0)
    ).reshape(128, 2 * B)
    in_maps = []
    for core in range(N_CORES):
        sh = n8[core * M_SHARD : (core + 1) * M_SHARD]  # [32768, 256]
        # PT8[p, c, t, j] = sh[c*2048 + j, t*128 + p]
        pt8 = np.ascontiguousarray(
            sh.reshape(N_CHUNKS, CHUNK, 2, 128).transpose(3, 0, 2, 1)
        ).reshape(128, 2 * M_SHARD)
        in_maps.append({"AT8": at8, "PT8": pt8})
    nc = _get_program()
    res = run_bass_kernel_spmd(
        nc, in_maps, core_ids=list(range(N_CORES)), trace=trace, tmpdir=tmpdir
    )
    cands = [res.results[c]["CAND"] for c in range(N_CORES)]
    return cands, res


def merge_loss(anchor, positive, cands):
    a = _normalize_rows(np.asarray(anchor, dtype=np.float32))
    p = _normalize_rows(np.asarray(positive, dtype=np.float32))
    pos_sim = (a * p).sum(axis=-1, dtype=np.float32) / TEMPERATURE  # [B]

    inv = 1.0 / (SCALE * SCALE)
    # A-region: cand[p, ((bi*16 + c)*2 + h)*8 + k] -> anchor bi*128+p
    a_parts = []  # [256, n] candidate sims
    y_parts = []  # [768, n]
    for cd in cands:
        cd = np.asarray(cd, dtype=np.float32)
        ar = cd[:, :A_REGION].reshape(128, N_A_B, N_CHUNKS * 2 * 8)
        a_parts.append(ar.transpose(1, 0, 2).reshape(N_A_B * 128, -1))
        yr = cd[:, A_REGION:]  # [128, 216]
        ys = []
        for byi in range(N_Y_B):
            o = (Y_CUM[byi]) * 8
            w = GROUPS_PER_B[byi] * 8
            ys.append(yr[:, o : o + w])
        # pad ragged groups to max width with -inf-ish (already -1e30 filled
        # only for written area; all group slots here are written)
        maxw = max(x.shape[1] for x in ys)
        ys = [
            np.pad(x, ((0, 0), (0, maxw - x.shape[1])), constant_values=-1e30)
            for x in ys
        ]
        y_parts.append(np.stack(ys, 0).reshape(N_Y_B * 128, -1))
    a_all = np.concatenate(a_parts, axis=1) * inv  # [256, 8*512/...] sims
    y_all = np.concatenate(y_parts, axis=1) * inv  # [768, ...]

    def topk(x, k):
        part = np.partition(x, x.shape[1] - k, axis=1)[:, -k:]
        return np.sort(part, axis=1)[:, ::-1]

    hard = np.concatenate(
        [topk(a_all, NUM_HARD_NEGATIVES), topk(y_all, NUM_HARD_NEGATIVES)],
        axis=0,
    ) / TEMPERATURE  # [B, 10] descending; rows ordered anchor 0..1023

    logits = np.concatenate([pos_sim[:, None], hard], axis=1).astype(np.float64)
    mx = logits.max(axis=1, keepdims=True)
    lse = mx[:, 0] + np.log(np.exp(logits - mx).sum(axis=1))
    loss = -(logits[:, 0] - lse).mean()
    return np.float32(loss)


def kernel(anchor, positive, negative_pool):
    cands, _ = run_device(anchor, negative_pool)
    return np.asarray(merge_loss(anchor, positive, cands), dtype=np.float32)



# revision 2
# speedup vs baseline: 1.3996x; 1.3996x over previous
"""HardNegativeInfoNCELoss on 8 Trainium2 NeuronCores.

Strategy (v3, exp-accumulate scan):
  * Host: L2-normalize anchor/positive/negative_pool (fp32), scale by 64 and
    quantize to fp8 e4m3 with the K=256 contraction packed as 2 k-tiles
    (DoubleRow).  Pool columns sharded across 8 cores (M/8 = 32768).
  * Device (SPMD, per core): stream the pool shard chunk-by-chunk (2048
    cols).  Per (128-anchor b-tile, 1024-col half-chunk) run 2 fp8 DR
    matmuls into a [128,1024] PSUM tile (4-deep rotation, PE ~216ns/MM).
    Each PSUM tile is consumed in ONE pass by one of two engines:
      - ScalarE: activation(Exp, scale=a, bias=-a*C) with accum_out ->
        acc = sum_j exp(a*(s_j - C)); the host recovers the window max as
        C + ln(acc)/a (exact to ~0.3 scaled units since the sum is
        max-dominated at a=0.11).  W=1024 windows.
      - VectorE: windowed tensor_reduce max [128,2,512] -> [128,2].
        W=512 windows.
    Both engines run ~1 elem/cycle; the scan is the critical path
    (~10.3us/chunk) with the matmul stream (6.9us/chunk) hidden under it.
  * Host: candidates = ACT lse-maxes + DVE window maxes (384 per row);
    exact top-10 per anchor, exact fp32 positive logit, InfoNCE loss.

  Window-collision safety: a window of W cols keeps only its max, losing a
  true top-10 member only when two land in one window (~9-18% of rows for
  W=512-1024); the lost member is replaced by rank 11 shifting the loss
  ~1e-4 relative.  Host-validated end to end: rel err 6.2e-4 vs fp32
  reference (gate 2e-2).
"""

import os
import sys

import numpy as np


def _ensure_concourse():
    try:
        import concourse  # noqa: F401
        return
    except ImportError:
        pass
    for p in ("/opt/trn_rl_repo", "/root/.axon_site/_ro/trn_rl_repo"):
        if os.path.isdir(os.path.join(p, "concourse")):
            sys.path.insert(0, p)
            return


_ensure_concourse()

N_CORES = 8
B = 1024
D = 256
M = 262144
M_SHARD = M // N_CORES  # 32768
CHUNK = 2048
N_CHUNKS = M_SHARD // CHUNK  # 16
NB = B // 128  # 8 anchor tiles
SCALE = 64.0
TEMPERATURE = 0.07
NUM_HARD_NEGATIVES = 10
EPS = 1e-12
ALPHA = 0.11
C_SHIFT = 1100.0

# tile (c, b, h): h=0 -> ACT exp-acc (W=1024), h=1 -> DVE reduce (2x W=512),
# except flipped tiles (c % 4 == 3, b == 7, h == 0) which go to DVE too,
# balancing ACT 124 : DVE 132 tiles per core.
FLIP_CS = [3, 7, 11, 15]
N_ACC = N_CHUNKS * NB              # 128 cols (4 never written -> 0)
N_RED = N_CHUNKS * NB * 2 + 8      # 264 cols

_program = None


def _is_flip(c, b):
    return (c % 4 == 3) and (b == 7)


def _build_program():
    import concourse.bacc as bacc
    import concourse.mybir as mybir
    from concourse.tile import TileContext

    nc = bacc.Bacc(
        "TRN2", target_bir_lowering=False, debug=False, num_devices=N_CORES
    )
    f32 = mybir.dt.float32
    fp8 = mybir.dt.float8e4
    DR = mybir.MatmulPerfMode.DoubleRow
    Exp = mybir.ActivationFunctionType.Exp
    Max = mybir.AluOpType.max
    X = mybir.AxisListType.X

    # AT8[p, t*1024 + m] = a8[m, t*128 + p];  PT8[p, c*4096 + t*2048 + j]
    AT8 = nc.dram_tensor("AT8", [128, 2 * B], fp8, kind="ExternalInput")
    PT8 = nc.dram_tensor("PT8", [128, 2 * M_SHARD], fp8, kind="ExternalInput")
    ACC = nc.dram_tensor("ACC", [128, N_ACC], f32, kind="ExternalOutput")
    RED = nc.dram_tensor("RED", [128, N_RED], f32, kind="ExternalOutput")

    with TileContext(nc) as tc:
        with (
            tc.tile_pool(name="const", bufs=1) as cpool,
            tc.tile_pool(name="stream", bufs=2) as spool,
            tc.tile_pool(name="psum", bufs=4, space="PSUM") as ppool,
        ):
            at8 = cpool.tile([128, 2 * B], fp8)
            nc.sync.dma_start(out=at8, in_=AT8[:, :])
            a8v = at8[:, :].rearrange("p (t m) -> p t m", t=2)  # [128,2,1024]

            biasap = cpool.tile([128, 1], f32)
            nc.vector.memset(biasap, -ALPHA * C_SHIFT)

            accb = cpool.tile([128, N_ACC], f32)
            nc.vector.memset(accb, 0.0)
            redb = cpool.tile([128, N_RED], f32)

            scratch = cpool.tile([128, 1024], f32)

            nflip = 0
            for c in range(N_CHUNKS):
                pt = spool.tile([128, 2 * CHUNK], fp8, tag="pt", name="pt")
                nc.sync.dma_start(
                    out=pt, in_=PT8[:, c * 2 * CHUNK: (c + 1) * 2 * CHUNK]
                )
                ptv = pt[:, :].rearrange("p (t j) -> p t j", t=2)

                for b in range(NB):
                    a_sl = a8v[:, :, b * 128: (b + 1) * 128]
                    for h in range(2):
                        ps = ppool.tile([128, 1024], f32, tag="ps", name="ps")
                        for n in range(2):
                            col = h * 1024 + n * 512
                            nc.tensor.matmul(
                                ps[:, n * 512: (n + 1) * 512],
                                a_sl,
                                ptv[:, :, col: col + 512],
                                start=True, stop=True, perf_mode=DR,
                            )
                        ti = c * NB + b
                        if h == 0 and not _is_flip(c, b):
                            nc.scalar.activation(
                                out=scratch, in_=ps, func=Exp,
                                bias=biasap[:, :], scale=ALPHA,
                                accum_out=accb[:, ti: ti + 1],
                            )
                        else:
                            if h == 1:
                                o = ti * 2
                            else:
                                o = N_CHUNKS * NB * 2 + 2 * nflip
                                nflip += 1
                            psw = ps[:, :].rearrange("p (g s) -> p g s", g=2)
                            nc.vector.tensor_reduce(
                                out=redb[:, o: o + 2], in_=psw, axis=X, op=Max,
                            )
            nc.sync.dma_start(out=ACC[:, :], in_=accb)
            nc.sync.dma_start(out=RED[:, :], in_=redb)
    nc.compile()
    return nc


def _get_program():
    global _program
    if _program is None:
        _program = _build_program()
    return _program


def _normalize_rows(x):
    n = np.sqrt((x.astype(np.float32) ** 2).sum(axis=-1, keepdims=True))
    return x / np.maximum(n, EPS)


def run_device(anchor, negative_pool, trace=False, tmpdir=None):
    """Run the SPMD device program; returns (per-core (ACC, RED) list, results)."""
    from concourse.bass_utils import run_bass_kernel_spmd

    import ml_dtypes

    fp8 = ml_dtypes.float8_e4m3
    a = _normalize_rows(np.asarray(anchor, dtype=np.float32)) * SCALE
    n = _normalize_rows(np.asarray(negative_pool, dtype=np.float32)) * SCALE
    a8 = a.astype(fp8)  # [B, 256]
    n8 = n.astype(fp8)  # [M, 256]

    # AT8[p, t, m] = a8[m, t*128+p]
    at8 = np.ascontiguousarray(
        a8.reshape(B, 2, 128).transpose(2, 1, 0)
    ).reshape(128, 2 * B)
    in_maps = []
    for core in range(N_CORES):
        sh = n8[core * M_SHARD: (core + 1) * M_SHARD]  # [32768, 256]
        # PT8[p, c, t, j] = sh[c*2048 + j, t*128 + p]
        pt8 = np.ascontiguousarray(
            sh.reshape(N_CHUNKS, CHUNK, 2, 128).transpose(3, 0, 2, 1)
        ).reshape(128, 2 * M_SHARD)
        in_maps.append({"AT8": at8, "PT8": pt8})
    nc = _get_program()
    res = run_bass_kernel_spmd(
        nc, in_maps, core_ids=list(range(N_CORES)), trace=trace, tmpdir=tmpdir
    )
    outs = [(res.results[c]["ACC"], res.results[c]["RED"])
            for c in range(N_CORES)]
    return outs, res


def merge_loss(anchor, positive, outs):
    a = _normalize_rows(np.asarray(anchor, dtype=np.float32))
    p = _normalize_rows(np.asarray(positive, dtype=np.float32))
    pos_sim = (a * p).sum(axis=-1, dtype=np.float32) / TEMPERATURE  # [B]

    inv = 1.0 / (SCALE * SCALE)
    parts = []  # per-core candidate arrays [B, ncand]
    for acc, red in outs:
        acc = np.asarray(acc, dtype=np.float32)  # [128, 128] cols = c*8+b
        red = np.asarray(red, dtype=np.float32)  # [128, 264]
        with np.errstate(divide="ignore"):
            lse = np.log(acc) / ALPHA + C_SHIFT  # -inf where acc == 0
        # candidates for row b*128+p live in acc[p, c*8+b], red[p, (c*8+b)*2+k]
        av = lse.reshape(128, N_CHUNKS, NB)          # [p, c, b]
        rv = red[:, : N_CHUNKS * NB * 2].reshape(128, N_CHUNKS, NB, 2)
        cand_b = []  # [b][128, ncand]
        for b in range(NB):
            cols = [av[:, :, b], rv[:, :, b, 0], rv[:, :, b, 1]]
            if b == 7:
                fl = red[:, N_CHUNKS * NB * 2:]      # [128, 8]
                cols.append(fl)
            cand_b.append(np.concatenate(cols, axis=1))
        w = max(x.shape[1] for x in cand_b)
        cand_b = [
            np.pad(x, ((0, 0), (0, w - x.shape[1])), constant_values=-np.inf)
            for x in cand_b
        ]
        parts.append(np.stack(cand_b, 0).reshape(B, -1))
    cand = np.concatenate(parts, axis=1) * inv / TEMPERATURE  # [B, ncand]
    cand = np.nan_to_num(cand, nan=-np.inf, posinf=-np.inf, neginf=-np.inf)

    k = NUM_HARD_NEGATIVES
    part = np.partition(cand, cand.shape[1] - k, axis=1)[:, -k:]
    hard = np.sort(part, axis=1)[:, ::-1]

    logits = np.concatenate([pos_sim[:, None], hard], axis=1).astype(np.float64)
    mx = logits.max(axis=1, keepdims=True)
    lse = mx[:, 0] + np.log(np.exp(logits - mx).sum(axis=1))
    loss = -(logits[:, 0] - lse).mean()
    return np.float32(loss)


def kernel(anchor, positive, negative_pool):
    outs, _ = run_device(anchor, negative_pool)
    return np.asarray(merge_loss(anchor, positive, outs), dtype=np.float32)


# revision 3
# speedup vs baseline: 1.4129x; 1.0095x over previous
"""HardNegativeInfoNCELoss on 8 Trainium2 NeuronCores.

Strategy (v3, exp-accumulate scan):
  * Host: L2-normalize anchor/positive/negative_pool (fp32), scale by 64 and
    quantize to fp8 e4m3 with the K=256 contraction packed as 2 k-tiles
    (DoubleRow).  Pool columns sharded across 8 cores (M/8 = 32768).
  * Device (SPMD, per core): stream the pool shard chunk-by-chunk (2048
    cols).  Per (128-anchor b-tile, 1024-col half-chunk) run 2 fp8 DR
    matmuls into a [128,1024] PSUM tile (4-deep rotation, PE ~216ns/MM).
    Each PSUM tile is consumed in ONE pass by one of two engines:
      - ScalarE: activation(Exp, scale=a, bias=-a*C) with accum_out ->
        acc = sum_j exp(a*(s_j - C)); the host recovers the window max as
        C + ln(acc)/a (exact to ~0.3 scaled units since the sum is
        max-dominated at a=0.11).  W=1024 windows.
      - VectorE: windowed tensor_reduce max [128,2,512] -> [128,2].
        W=512 windows.
    Both engines run ~1 elem/cycle; the scan is the critical path
    (~10.3us/chunk) with the matmul stream (6.9us/chunk) hidden under it.
  * Host: candidates = ACT lse-maxes + DVE window maxes (384 per row);
    exact top-10 per anchor, exact fp32 positive logit, InfoNCE loss.

  Window-collision safety: a window of W cols keeps only its max, losing a
  true top-10 member only when two land in one window (~9-18% of rows for
  W=512-1024); the lost member is replaced by rank 11 shifting the loss
  ~1e-4 relative.  Host-validated end to end: rel err 6.2e-4 vs fp32
  reference (gate 2e-2).
"""

import os
import sys

import numpy as np


def _ensure_concourse():
    try:
        import concourse  # noqa: F401
        return
    except ImportError:
        pass
    for p in ("/opt/trn_rl_repo", "/root/.axon_site/_ro/trn_rl_repo"):
        if os.path.isdir(os.path.join(p, "concourse")):
            sys.path.insert(0, p)
            return


_ensure_concourse()

N_CORES = 8
B = 1024
D = 256
M = 262144
M_SHARD = M // N_CORES  # 32768
CHUNK = 2048
N_CHUNKS = M_SHARD // CHUNK  # 16
NB = B // 128  # 8 anchor tiles
SCALE = 64.0
TEMPERATURE = 0.07
NUM_HARD_NEGATIVES = 10
EPS = 1e-12
ALPHA = 0.11
C_SHIFT = 1100.0

# tile (c, b, h): h=0 -> ACT exp-acc (W=1024), h=1 -> DVE reduce (2x W=512),
# except flipped tiles (c odd, b == 7, h == 0) which go to DVE too,
# balancing ACT 120 : DVE 136 tiles per core (ACT ~1330ns/tile vs DVE ~1175).
N_FLIP = 8
N_ACC = N_CHUNKS * NB              # 128 cols (8 never written -> 0)
N_RED = N_CHUNKS * NB * 2 + 2 * N_FLIP  # 272 cols

_program = None


def _is_flip(c, b):
    return (c % 2 == 1) and (b == 7)


def _build_program():
    import concourse.bacc as bacc
    import concourse.mybir as mybir
    from concourse.tile import TileContext

    nc = bacc.Bacc(
        "TRN2", target_bir_lowering=False, debug=False, num_devices=N_CORES
    )
    f32 = mybir.dt.float32
    fp8 = mybir.dt.float8e4
    DR = mybir.MatmulPerfMode.DoubleRow
    Exp = mybir.ActivationFunctionType.Exp
    Max = mybir.AluOpType.max
    X = mybir.AxisListType.X

    # AT8[p, t*1024 + m] = a8[m, t*128 + p];  PT8[p, c*4096 + t*2048 + j]
    AT8 = nc.dram_tensor("AT8", [128, 2 * B], fp8, kind="ExternalInput")
    PT8 = nc.dram_tensor("PT8", [128, 2 * M_SHARD], fp8, kind="ExternalInput")
    ACC = nc.dram_tensor("ACC", [128, N_ACC], f32, kind="ExternalOutput")
    RED = nc.dram_tensor("RED", [128, N_RED], f32, kind="ExternalOutput")

    with TileContext(nc) as tc:
        with (
            tc.tile_pool(name="const", bufs=1) as cpool,
            tc.tile_pool(name="stream", bufs=2) as spool,
            tc.tile_pool(name="psum", bufs=4, space="PSUM") as ppool,
        ):
            at8 = cpool.tile([128, 2 * B], fp8)
            nc.sync.dma_start(out=at8, in_=AT8[:, :])
            a8v = at8[:, :].rearrange("p (t m) -> p t m", t=2)  # [128,2,1024]

            biasap = cpool.tile([128, 1], f32)
            nc.vector.memset(biasap, -ALPHA * C_SHIFT)

            accb = cpool.tile([128, N_ACC], f32)
            nc.vector.memset(accb, 0.0)
            redb = cpool.tile([128, N_RED], f32)

            scratch = cpool.tile([128, 1024], f32)

            nflip = 0
            for c in range(N_CHUNKS):
                pt = spool.tile([128, 2 * CHUNK], fp8, tag="pt", name="pt")
                nc.sync.dma_start(
                    out=pt, in_=PT8[:, c * 2 * CHUNK: (c + 1) * 2 * CHUNK]
                )
                ptv = pt[:, :].rearrange("p (t j) -> p t j", t=2)

                for b in range(NB):
                    a_sl = a8v[:, :, b * 128: (b + 1) * 128]
                    for h in range(2):
                        ps = ppool.tile([128, 1024], f32, tag="ps", name="ps")
                        for n in range(2):
                            col = h * 1024 + n * 512
                            nc.tensor.matmul(
                                ps[:, n * 512: (n + 1) * 512],
                                a_sl,
                                ptv[:, :, col: col + 512],
                                start=True, stop=True, perf_mode=DR,
                            )
                        ti = c * NB + b
                        if h == 0 and not _is_flip(c, b):
                            nc.scalar.activation(
                                out=scratch, in_=ps, func=Exp,
                                bias=biasap[:, :], scale=ALPHA,
                                accum_out=accb[:, ti: ti + 1],
                            )
                        else:
                            if h == 1:
                                o = ti * 2
                            else:
                                o = N_CHUNKS * NB * 2 + 2 * nflip
                                nflip += 1
                            psw = ps[:, :].rearrange("p (g s) -> p g s", g=2)
                            nc.vector.tensor_reduce(
                                out=redb[:, o: o + 2], in_=psw, axis=X, op=Max,
                            )
            nc.sync.dma_start(out=ACC[:, :], in_=accb)
            nc.sync.dma_start(out=RED[:, :], in_=redb)
    nc.compile()
    return nc


def _get_program():
    global _program
    if _program is None:
        _program = _build_program()
    return _program


def _normalize_rows(x):
    n = np.sqrt((x.astype(np.float32) ** 2).sum(axis=-1, keepdims=True))
    return x / np.maximum(n, EPS)


def run_device(anchor, negative_pool, trace=False, tmpdir=None):
    """Run the SPMD device program; returns (per-core (ACC, RED) list, results)."""
    from concourse.bass_utils import run_bass_kernel_spmd

    import ml_dtypes

    fp8 = ml_dtypes.float8_e4m3
    a = _normalize_rows(np.asarray(anchor, dtype=np.float32)) * SCALE
    n = _normalize_rows(np.asarray(negative_pool, dtype=np.float32)) * SCALE
    a8 = a.astype(fp8)  # [B, 256]
    n8 = n.astype(fp8)  # [M, 256]

    # AT8[p, t, m] = a8[m, t*128+p]
    at8 = np.ascontiguousarray(
        a8.reshape(B, 2, 128).transpose(2, 1, 0)
    ).reshape(128, 2 * B)
    in_maps = []
    for core in range(N_CORES):
        sh = n8[core * M_SHARD: (core + 1) * M_SHARD]  # [32768, 256]
        # PT8[p, c, t, j] = sh[c*2048 + j, t*128 + p]
        pt8 = np.ascontiguousarray(
            sh.reshape(N_CHUNKS, CHUNK, 2, 128).transpose(3, 0, 2, 1)
        ).reshape(128, 2 * M_SHARD)
        in_maps.append({"AT8": at8, "PT8": pt8})
    nc = _get_program()
    res = run_bass_kernel_spmd(
        nc, in_maps, core_ids=list(range(N_CORES)), trace=trace, tmpdir=tmpdir
    )
    outs = [(res.results[c]["ACC"], res.results[c]["RED"])
            for c in range(N_CORES)]
    return outs, res


def merge_loss(anchor, positive, outs):
    a = _normalize_rows(np.asarray(anchor, dtype=np.float32))
    p = _normalize_rows(np.asarray(positive, dtype=np.float32))
    pos_sim = (a * p).sum(axis=-1, dtype=np.float32) / TEMPERATURE  # [B]

    inv = 1.0 / (SCALE * SCALE)
    parts = []  # per-core candidate arrays [B, ncand]
    for acc, red in outs:
        acc = np.asarray(acc, dtype=np.float32)  # [128, 128] cols = c*8+b
        red = np.asarray(red, dtype=np.float32)  # [128, 264]
        with np.errstate(divide="ignore"):
            lse = np.log(acc) / ALPHA + C_SHIFT  # -inf where acc == 0
        # candidates for row b*128+p live in acc[p, c*8+b], red[p, (c*8+b)*2+k]
        av = lse.reshape(128, N_CHUNKS, NB)          # [p, c, b]
        rv = red[:, : N_CHUNKS * NB * 2].reshape(128, N_CHUNKS, NB, 2)
        cand_b = []  # [b][128, ncand]
        for b in range(NB):
            cols = [av[:, :, b], rv[:, :, b, 0], rv[:, :, b, 1]]
            if b == 7:
                fl = red[:, N_CHUNKS * NB * 2:]      # [128, 8]
                cols.append(fl)
            cand_b.append(np.concatenate(cols, axis=1))
        w = max(x.shape[1] for x in cand_b)
        cand_b = [
            np.pad(x, ((0, 0), (0, w - x.shape[1])), constant_values=-np.inf)
            for x in cand_b
        ]
        parts.append(np.stack(cand_b, 0).reshape(B, -1))
    cand = np.concatenate(parts, axis=1) * inv / TEMPERATURE  # [B, ncand]
    cand = np.nan_to_num(cand, nan=-np.inf, posinf=-np.inf, neginf=-np.inf)

    k = NUM_HARD_NEGATIVES
    part = np.partition(cand, cand.shape[1] - k, axis=1)[:, -k:]
    hard = np.sort(part, axis=1)[:, ::-1]

    logits = np.concatenate([pos_sim[:, None], hard], axis=1).astype(np.float64)
    mx = logits.max(axis=1, keepdims=True)
    lse = mx[:, 0] + np.log(np.exp(logits - mx).sum(axis=1))
    loss = -(logits[:, 0] - lse).mean()
    return np.float32(loss)


def kernel(anchor, positive, negative_pool):
    outs, _ = run_device(anchor, negative_pool)
    return np.asarray(merge_loss(anchor, positive, outs), dtype=np.float32)
